# revision 4
# baseline (speedup 1.0000x reference)
"""DotProductAttentionPooling on 8 trn2 NeuronCores.

reference:
    scores = einsum("bld,d->bl", x, q) / sqrt(D)
    scores = where(mask, scores, -inf)
    attn   = nan_to_num(softmax(scores, axis=-1))
    out    = einsum("bl,bld->bd", attn, x)            # [B, D]

Strategy (memory-bound: x is 256 MiB and must be read exactly once):
  - Data-parallel: batch B=32 sharded 4-per-core across 8 cores; query
    replicated; output [B, D] gathered on host.
  - x[b] streams to SBUF in natural layout [128(L-part), chunk, 256(D)]
    with l = p*64 + i so each partition's HBM read is contiguous.
    Processing unit is a "block" of 16 chunks (2 MiB fp32), fetched as
    two 1 MiB dma_starts on the sync HWDGE ring, LOOKAHEAD blocks deep.
  - fp32->fp16 convert is split between ScalarE (chunks [0,C_SE)) and
    GpSimd (the rest) and issued one block ahead of its consumers, so
    the fp32 tile is freed at convert rate and the ScalarE queue never
    serializes behind the score/exp consumers.
  - Scores are split to keep every engine under the DMA roofline (DVE
    STT is 1x-rate only, DVE alone cannot keep up): chunks [0, A_DV)
    run as DVE STT with fp32 accum_out; chunks [A_DV, 16) get their
    fp16 product from one slab tensor_tensor on DVE (2x rate, one op
    amortizes overheads; query broadcast via stride-0 AP) and are then
    reduced on ScalarE as activation Copy+accum_out, one per chunk.
  - Softmax without max-subtraction: scores are O(0.3) so exp cannot
    overflow; the -inf mask becomes w = exp(scores) * mask. exp, mask
    multiply and pooling run per block so pooling starts before the
    batch finishes; denominator = ones-matmul over the per-block
    row-sums, +1e-30 so an all-masked batch yields 0 (like
    nan_to_num), not NaN.
  - Pooling: unnormalized acc[1, 256] += w_col.T @ x_chunk as fp16
    accumulating PE matmuls (contraction over partition dim = L);
    final normalize on ScalarE out of PSUM, deferred one batch so it
    never stalls the pipeline.
"""

import numpy as np

B, L, D = 32, 8192, 256
N_CORES = 8
BPC = B // N_CORES        # batches per core
P = 128                   # partitions
CHUNKS = L // P           # 64 L-chunks per batch
BC = 16                   # chunks per block
NB = CHUNKS // BC         # blocks per batch (4)
NBT = BPC * NB            # total blocks per core (16)
LOOKAHEAD = 6             # blocks of DMA prefetch (fp32 bufs)
A_DV = 10                 # chunks per block scored as fused DVE STT
C_SE = 10                 # chunks per block converted on ScalarE (rest GpSimd)
SCALE = 1.0 / float(np.sqrt(D))

_cache = {}


def _build():
    import concourse.bacc as bacc
    import concourse.bass as bass
    import concourse.tile as tile
    from concourse import mybir

    f32 = mybir.dt.float32
    f16 = mybir.dt.float16
    i32 = mybir.dt.int32
    nc = bacc.Bacc("TRN2", target_bir_lowering=False, debug=False,
                   num_devices=N_CORES)

    x = nc.declare_dram_parameter("x", [BPC, L, D], f32, isOutput=False)
    mask = nc.declare_dram_parameter("mask", [BPC, L], i32, isOutput=False)
    query = nc.declare_dram_parameter("query", [D], f32, isOutput=False)
    out = nc.declare_dram_parameter("out", [BPC, D], f32, isOutput=True)

    # l = p * CHUNKS + i: per-partition HBM reads are contiguous
    x_r = x[:].rearrange("b (p i) d -> b p i d", p=P)
    mask_r = mask[:].rearrange("b (p i) -> b p i", p=P)
    N_OFF = BC - A_DV       # chunks reduced on ScalarE

    with tile.TileContext(nc) as tc:
        with (
            tc.tile_pool(name="xf32", bufs=LOOKAHEAD) as xf32p,
            tc.tile_pool(name="xf16", bufs=4) as xf16p,
            tc.tile_pool(name="small", bufs=4) as small,
            tc.tile_pool(name="scratch", bufs=2) as scratchp,
            tc.tile_pool(name="prod", bufs=2) as prodp,
            tc.tile_pool(name="singles", bufs=1) as singles,
            tc.tile_pool(name="psum", bufs=2, space="PSUM") as psums,
        ):
            # broadcast query across partitions with a step-0 SWDGE DMA,
            # issued first so it lands before the first score op needs it
            qb = singles.tile([P, D], f32)
            q_ap = query[:]
            nc.gpsimd.dma_start(out=qb[:], in_=bass.AP(
                tensor=q_ap.tensor, offset=q_ap.offset,
                ap=[[0, P]] + list(q_ap.ap)))
            qh = singles.tile([P, D], f16)
            nc.scalar.copy(qh[:], qb[:])
            ones = singles.tile([P, 1], f32)
            nc.vector.memset(ones[:], 1.0)

            def qh_bcast(n):
                # [P, n, D] view of qh with stride-0 chunk dim
                a = qh[:]
                return bass.AP(tensor=a.tensor, offset=a.offset,
                               ap=[list(a.ap[0]), [0, n], list(a.ap[1])])

            xq_tiles = {}       # block index -> staged fp32 tile
            xh_tiles = {}       # block index -> fp16 copy
            mask_tiles = {}     # batch -> int32 mask tile
            state = {}          # per-batch softmax state
            epilogue = []       # deferred (pool_ps, wsum, b)

            def issue_block(k):
                b, qi = divmod(k, NB)
                if qi == 0 and b not in mask_tiles:
                    mi = small.tile([P, CHUNKS], i32, tag="mask_i")
                    nc.sync.dma_start(out=mi[:], in_=mask_r[b])
                    mask_tiles[b] = mi
                xq = xf32p.tile([P, BC, D], f32, tag="xf")
                ndma = 4 if k == 0 else 2
                step = BC // ndma
                for g in range(ndma):
                    sl = slice(qi * BC + g * step, qi * BC + (g + 1) * step)
                    nc.sync.dma_start(out=xq[:, g * step:(g + 1) * step, :],
                                      in_=x_r[b, :, sl, :])
                xq_tiles[k] = xq

            def convert_block(k):
                xq = xq_tiles.pop(k)
                xh = xf16p.tile([P, BC, D], f16, tag="xh", name=f"xh{k}")
                nc.scalar.copy(xh[:, 0:C_SE, :], xq[:, 0:C_SE, :])
                nc.gpsimd.tensor_copy(xh[:, C_SE:BC, :], xq[:, C_SE:BC, :])
                xh_tiles[k] = xh

            def flush_epilogue():
                while epilogue:
                    pool_ps, wsum, bb = epilogue.pop()
                    den_ps = psums.tile([1, NB], f32, tag="den",
                                        name=f"den_ps{bb}")
                    nc.tensor.matmul(den_ps[:], ones[:], wsum[:],
                                     start=True, stop=True)
                    den_sb = small.tile([1, 1], f32, tag="den_sb",
                                        name=f"den_sb{bb}")
                    nc.vector.tensor_reduce(out=den_sb[:], in_=den_ps[:],
                                            op=mybir.AluOpType.add,
                                            axis=mybir.AxisListType.X)
                    # +1e-30 so an all-masked batch divides to 0, not NaN
                    den_eps = small.tile([1, 1], f32, tag="den_eps",
                                         name=f"den_eps{bb}")
                    nc.vector.tensor_scalar_add(den_eps[:], den_sb[:], 1e-30)
                    rden = small.tile([1, 1], f32, tag="rden",
                                      name=f"rden{bb}")
                    nc.vector.reciprocal(rden[:], den_eps[:])
                    out_sb = small.tile([1, D], f32)
                    nc.scalar.activation(
                        out=out_sb[:], in_=pool_ps[:],
                        func=mybir.ActivationFunctionType.Copy,
                        scale=rden[0:1, 0:1])
                    nc.sync.dma_start(out=out[bb:bb + 1, :], in_=out_sb[:])

            for k in range(min(LOOKAHEAD, NBT)):
                issue_block(k)
            convert_block(0)

            for k in range(NBT):
                b, qi = divmod(k, NB)
                if qi == 0:
                    state[b] = {
                        "scores": small.tile([P, CHUNKS], f32, tag="scores",
                                             name=f"scores{b}"),
                        "wsum": small.tile([P, NB], f32, tag="wsum",
                                           name=f"wsum_{b}"),
                        "pool_ps": psums.tile([1, D], f32, tag="pool",
                                              name=f"pool_ps{b}"),
                        "mask_f": small.tile([P, CHUNKS], f32, tag="mask_f",
                                             name=f"mask_f{b}"),
                    }
                st = state[b]
                xh = xh_tiles.pop(k)

                # fp16 product slab for the ScalarE-reduced chunks, issued
                # first on DVE so ScalarE's reduces can start early
                if N_OFF:
                    prodh = prodp.tile([P, N_OFF, D], f16, tag="prodh")
                    nc.vector.tensor_tensor(
                        out=prodh[:], in0=xh[:, A_DV:BC, :],
                        in1=qh_bcast(N_OFF), op=mybir.AluOpType.mult)

                # convert block k+1 now so the ScalarE/GpSimd queues stay
                # one block ahead of the score/exp consumers
                if k + 1 < NBT:
                    convert_block(k + 1)
                if k + LOOKAHEAD < NBT:
                    issue_block(k + LOOKAHEAD)
                if qi == 0:
                    nc.vector.tensor_copy(st["mask_f"][:],
                                          mask_tiles.pop(b)[:])

                if b > 0 and qi == 0:
                    flush_epilogue()

                # ScalarE reduces for the slab chunks
                for j in range(N_OFF):
                    col = st["scores"][:, qi * BC + A_DV + j:
                                       qi * BC + A_DV + j + 1]
                    scrh = scratchp.tile([P, D], f16, tag="scrh")
                    nc.scalar.activation(
                        out=scrh[:], in_=prodh[:, j, :],
                        func=mybir.ActivationFunctionType.Copy,
                        scale=SCALE, accum_out=col)

                # DVE fused STT for the rest
                for i in range(A_DV):
                    col = st["scores"][:, qi * BC + i:qi * BC + i + 1]
                    scr = scratchp.tile([P, D], f16, tag="scr")
                    nc.vector.scalar_tensor_tensor(
                        out=scr[:],
                        in0=xh[:, i, :],
                        scalar=SCALE,
                        in1=qh[:],
                        op0=mybir.AluOpType.mult,
                        op1=mybir.AluOpType.mult,
                        accum_out=col,
                    )

                # per-block softmax tail: exp, mask, fp16 weights
                sl = slice(qi * BC, (qi + 1) * BC)
                expq = small.tile([P, BC], f32, tag="expq")
                nc.scalar.activation(out=expq[:], in_=st["scores"][:, sl],
                                     func=mybir.ActivationFunctionType.Exp)
                wqh = small.tile([P, BC], f16, tag="wqh")
                nc.vector.scalar_tensor_tensor(
                    out=wqh[:], in0=expq[:], scalar=1.0,
                    in1=st["mask_f"][:, sl],
                    op0=mybir.AluOpType.mult, op1=mybir.AluOpType.mult,
                    accum_out=st["wsum"][:, qi:qi + 1],
                )

                for i in range(BC):
                    nc.tensor.matmul(
                        st["pool_ps"][:],
                        wqh[:, i:i + 1],
                        xh[:, i, :],
                        start=(qi == 0 and i == 0),
                        stop=(qi == NB - 1 and i == BC - 1),
                    )

                if qi == NB - 1:
                    epilogue.append((st["pool_ps"], st["wsum"], b))
                    del state[b]

            flush_epilogue()

    nc.compile()
    return nc


def kernel(x: np.ndarray, mask: np.ndarray, query: np.ndarray) -> np.ndarray:
    from concourse.bass_utils import run_bass_kernel_spmd

    if "nc" not in _cache:
        _cache["nc"] = _build()
    nc = _cache["nc"]

    x = np.ascontiguousarray(np.asarray(x, dtype=np.float32))
    mask = np.ascontiguousarray(np.asarray(mask, dtype=np.int32))
    query = np.ascontiguousarray(np.asarray(query, dtype=np.float32))

    in_maps = [
        {
            "x": np.ascontiguousarray(x[c * BPC:(c + 1) * BPC]),
            "mask": np.ascontiguousarray(mask[c * BPC:(c + 1) * BPC]),
            "query": query,
        }
        for c in range(N_CORES)
    ]
    res = run_bass_kernel_spmd(nc, in_maps, core_ids=list(range(N_CORES)))
    return np.concatenate([res.results[c]["out"] for c in range(N_CORES)], axis=0)


# revision 5
# speedup vs baseline: 1.6139x; 1.6139x over previous
"""DotProductAttentionPooling on 8 trn2 NeuronCores.

reference:
    scores = einsum("bld,d->bl", x, q) / sqrt(D)
    scores = where(mask, scores, -inf)
    attn   = nan_to_num(softmax(scores, axis=-1))
    out    = einsum("bl,bld->bd", attn, x)            # [B, D]

Strategy (memory-bound: x is 256 MiB and must be read exactly once):
  - Data-parallel: batch B=32 sharded 4-per-core across 8 cores; query
    replicated; output [B, D] gathered on host.
  - x[b] streams to SBUF in natural layout [128(L-part), chunk, 256(D)]
    with l = p*64 + i so each partition's HBM read is contiguous.
    Processing unit is a "block" of 16 chunks (2 MiB fp32), fetched as
    two 1 MiB dma_starts on the sync HWDGE ring, LOOKAHEAD blocks deep.
  - ScalarE converts each block to fp16 in one [128,4096] activation,
    issued one block ahead of its consumers; the fp32 tile is freed at
    convert rate so DMA never waits on the score pipeline.
  - Scores per block run entirely on DVE but avoid the 1x-rate fused
    STT: one slab tensor_tensor product (fp16 2x rate, query broadcast
    via stride-0 AP), then a tree of fp16 tensor_tensor adds (each 2x)
    folding D 256->32, then one segmented tensor_reduce [128,16,32] ->
    fp32 score columns. ~5.4us/block vs ~6.7us for 16 fused STTs.
  - Softmax without max-subtraction: scores are O(0.3) so exp cannot
    overflow; 1/sqrt(D) is folded into exp's activation scale; the
    -inf mask becomes w = exp(scores) * mask. exp, mask multiply and
    pooling run per block so pooling starts before the batch finishes;
    denominator = ones-matmul over the per-block row-sums, +1e-30 so
    an all-masked batch yields 0 (like nan_to_num), not NaN.
  - Pooling: unnormalized acc[1, 256] += w_col.T @ x_chunk as fp16
    accumulating PE matmuls (contraction over partition dim = L);
    final normalize on ScalarE out of PSUM, deferred one batch so it
    never stalls the pipeline.
"""

import numpy as np

B, L, D = 32, 8192, 256
N_CORES = 8
BPC = B // N_CORES        # batches per core
P = 128                   # partitions
CHUNKS = L // P           # 64 L-chunks per batch
BC = 16                   # chunks per block
NB = CHUNKS // BC         # blocks per batch (4)
NBT = BPC * NB            # total blocks per core (16)
LOOKAHEAD = 6             # blocks of DMA prefetch (fp32 bufs)
FOLD_STOP = 32            # tree-fold D down to this width, then reduce
SCALE = 1.0 / float(np.sqrt(D))

_cache = {}


def _build():
    import concourse.bacc as bacc
    import concourse.bass as bass
    import concourse.tile as tile
    from concourse import mybir

    f32 = mybir.dt.float32
    f16 = mybir.dt.float16
    i32 = mybir.dt.int32
    nc = bacc.Bacc("TRN2", target_bir_lowering=False, debug=False,
                   num_devices=N_CORES)

    x = nc.declare_dram_parameter("x", [BPC, L, D], f32, isOutput=False)
    mask = nc.declare_dram_parameter("mask", [BPC, L], i32, isOutput=False)
    query = nc.declare_dram_parameter("query", [D], f32, isOutput=False)
    out = nc.declare_dram_parameter("out", [BPC, D], f32, isOutput=True)

    # l = p * CHUNKS + i: per-partition HBM reads are contiguous
    x_r = x[:].rearrange("b (p i) d -> b p i d", p=P)
    mask_r = mask[:].rearrange("b (p i) -> b p i", p=P)

    with tile.TileContext(nc) as tc:
        with (
            tc.tile_pool(name="xf32", bufs=LOOKAHEAD) as xf32p,
            tc.tile_pool(name="xf16", bufs=4) as xf16p,
            tc.tile_pool(name="small", bufs=4) as small,
            tc.tile_pool(name="prod", bufs=2) as prodp,
            tc.tile_pool(name="fold", bufs=2) as foldp,
            tc.tile_pool(name="singles", bufs=1) as singles,
            tc.tile_pool(name="psum", bufs=2, space="PSUM") as psums,
        ):
            # broadcast query across partitions with a step-0 SWDGE DMA,
            # issued first so it lands before the first score op needs it
            qb = singles.tile([P, D], f32)
            q_ap = query[:]
            nc.gpsimd.dma_start(out=qb[:], in_=bass.AP(
                tensor=q_ap.tensor, offset=q_ap.offset,
                ap=[[0, P]] + list(q_ap.ap)))
            qh = singles.tile([P, D], f16)
            nc.scalar.copy(qh[:], qb[:])
            ones = singles.tile([P, 1], f32)
            nc.vector.memset(ones[:], 1.0)

            def qh_bcast(n):
                # [P, n, D] view of qh with stride-0 chunk dim
                a = qh[:]
                return bass.AP(tensor=a.tensor, offset=a.offset,
                               ap=[list(a.ap[0]), [0, n], list(a.ap[1])])

            xq_tiles = {}       # block index -> staged fp32 tile
            xh_tiles = {}       # block index -> fp16 copy
            mask_tiles = {}     # batch -> int32 mask tile
            state = {}          # per-batch softmax state
            epilogue = []       # deferred (pool_ps, wsum, b)

            def issue_block(k):
                b, qi = divmod(k, NB)
                if qi == 0 and b not in mask_tiles:
                    mi = small.tile([P, CHUNKS], i32, tag="mask_i")
                    nc.sync.dma_start(out=mi[:], in_=mask_r[b])
                    mask_tiles[b] = mi
                xq = xf32p.tile([P, BC, D], f32, tag="xf")
                ndma = 4 if k == 0 else 2
                step = BC // ndma
                for g in range(ndma):
                    sl = slice(qi * BC + g * step, qi * BC + (g + 1) * step)
                    nc.sync.dma_start(out=xq[:, g * step:(g + 1) * step, :],
                                      in_=x_r[b, :, sl, :])
                xq_tiles[k] = xq

            def convert_block(k):
                xq = xq_tiles.pop(k)
                xh = xf16p.tile([P, BC, D], f16, tag="xh", name=f"xh{k}")
                nc.scalar.copy(xh[:], xq[:])
                xh_tiles[k] = xh

            def flush_epilogue():
                while epilogue:
                    pool_ps, wsum, bb = epilogue.pop()
                    den_ps = psums.tile([1, NB], f32, tag="den",
                                        name=f"den_ps{bb}")
                    nc.tensor.matmul(den_ps[:], ones[:], wsum[:],
                                     start=True, stop=True)
                    den_sb = small.tile([1, 1], f32, tag="den_sb",
                                        name=f"den_sb{bb}")
                    nc.vector.tensor_reduce(out=den_sb[:], in_=den_ps[:],
                                            op=mybir.AluOpType.add,
                                            axis=mybir.AxisListType.X)
                    # +1e-30 so an all-masked batch divides to 0, not NaN
                    den_eps = small.tile([1, 1], f32, tag="den_eps",
                                         name=f"den_eps{bb}")
                    nc.vector.tensor_scalar_add(den_eps[:], den_sb[:], 1e-30)
                    rden = small.tile([1, 1], f32, tag="rden",
                                      name=f"rden{bb}")
                    nc.vector.reciprocal(rden[:], den_eps[:])
                    out_sb = small.tile([1, D], f32)
                    nc.scalar.activation(
                        out=out_sb[:], in_=pool_ps[:],
                        func=mybir.ActivationFunctionType.Copy,
                        scale=rden[0:1, 0:1])
                    nc.sync.dma_start(out=out[bb:bb + 1, :], in_=out_sb[:])

            for k in range(min(LOOKAHEAD, NBT)):
                issue_block(k)
            convert_block(0)

            for k in range(NBT):
                b, qi = divmod(k, NB)
                if qi == 0:
                    state[b] = {
                        "scores": small.tile([P, CHUNKS], f32, tag="scores",
                                             name=f"scores{b}"),
                        "wsum": small.tile([P, NB], f32, tag="wsum",
                                           name=f"wsum_{b}"),
                        "pool_ps": psums.tile([1, D], f32, tag="pool",
                                              name=f"pool_ps{b}"),
                        "mask_f": small.tile([P, CHUNKS], f32, tag="mask_f",
                                             name=f"mask_f{b}"),
                    }
                st = state[b]
                xh = xh_tiles.pop(k)

                # fp16 product slab, first on the DVE queue
                prodh = prodp.tile([P, BC, D], f16, tag="prodh")
                nc.vector.tensor_tensor(
                    out=prodh[:], in0=xh[:], in1=qh_bcast(BC),
                    op=mybir.AluOpType.mult)

                # convert block k+1 so ScalarE stays one block ahead
                if k + 1 < NBT:
                    convert_block(k + 1)
                if k + LOOKAHEAD < NBT:
                    issue_block(k + LOOKAHEAD)
                if qi == 0:
                    nc.vector.tensor_copy(st["mask_f"][:],
                                          mask_tiles.pop(b)[:])

                if b > 0 and qi == 0:
                    flush_epilogue()

                # fp16 tree-fold of D: 256 -> FOLD_STOP, all 2x-rate adds
                cur, w = prodh, D
                while w > FOLD_STOP:
                    h = w // 2
                    nxt = foldp.tile([P, BC, h], f16, tag=f"fold{h}")
                    nc.vector.tensor_tensor(
                        out=nxt[:], in0=cur[:, :, 0:h], in1=cur[:, :, h:w],
                        op=mybir.AluOpType.add)
                    cur, w = nxt, h

                # segmented reduce -> fp32 score columns of this block
                sl = slice(qi * BC, (qi + 1) * BC)
                nc.vector.tensor_reduce(
                    out=st["scores"][:, sl], in_=cur[:],
                    op=mybir.AluOpType.add, axis=mybir.AxisListType.X)

                # per-block softmax tail: exp (with 1/sqrt(D) folded into
                # the activation scale), mask multiply, fp16 weights
                expq = small.tile([P, BC], f32, tag="expq")
                nc.scalar.activation(out=expq[:], in_=st["scores"][:, sl],
                                     func=mybir.ActivationFunctionType.Exp,
                                     scale=SCALE)
                wqh = small.tile([P, BC], f16, tag="wqh")
                nc.vector.scalar_tensor_tensor(
                    out=wqh[:], in0=expq[:], scalar=1.0,
                    in1=st["mask_f"][:, sl],
                    op0=mybir.AluOpType.mult, op1=mybir.AluOpType.mult,
                    accum_out=st["wsum"][:, qi:qi + 1],
                )

                for i in range(BC):
                    nc.tensor.matmul(
                        st["pool_ps"][:],
                        wqh[:, i:i + 1],
                        xh[:, i, :],
                        start=(qi == 0 and i == 0),
                        stop=(qi == NB - 1 and i == BC - 1),
                    )

                if qi == NB - 1:
                    epilogue.append((st["pool_ps"], st["wsum"], b))
                    del state[b]

            flush_epilogue()

    nc.compile()
    return nc


def kernel(x: np.ndarray, mask: np.ndarray, query: np.ndarray) -> np.ndarray:
    from concourse.bass_utils import run_bass_kernel_spmd

    if "nc" not in _cache:
        _cache["nc"] = _build()
    nc = _cache["nc"]

    x = np.ascontiguousarray(np.asarray(x, dtype=np.float32))
    mask = np.ascontiguousarray(np.asarray(mask, dtype=np.int32))
    query = np.ascontiguousarray(np.asarray(query, dtype=np.float32))

    in_maps = [
        {
            "x": np.ascontiguousarray(x[c * BPC:(c + 1) * BPC]),
            "mask": np.ascontiguousarray(mask[c * BPC:(c + 1) * BPC]),
            "query": query,
        }
        for c in range(N_CORES)
    ]
    res = run_bass_kernel_spmd(nc, in_maps, core_ids=list(range(N_CORES)))
    return np.concatenate([res.results[c]["out"] for c in range(N_CORES)], axis=0)


# revision 9
# speedup vs baseline: 1.6454x; 1.0195x over previous
"""DotProductAttentionPooling on 8 trn2 NeuronCores.

reference:
    scores = einsum("bld,d->bl", x, q) / sqrt(D)
    scores = where(mask, scores, -inf)
    attn   = nan_to_num(softmax(scores, axis=-1))
    out    = einsum("bl,bld->bd", attn, x)            # [B, D]

Strategy (memory-bound: x is 256 MiB and must be read exactly once):
  - Data-parallel: batch B=32 sharded 4-per-core across 8 cores; query
    replicated; output [B, D] gathered on host.
  - x[b] streams to SBUF in natural layout [128(L-part), chunk, 256(D)]
    with l = p*64 + i so each partition's HBM read is contiguous.
    Processing unit is a "block" of 16 chunks (2 MiB fp32), fetched as
    two 1 MiB dma_starts on the sync HWDGE ring, LOOKAHEAD blocks deep.
  - ScalarE converts each block to fp16 in one [128,4096] activation,
    issued one block ahead of its consumers; the fp32 tile is freed at
    convert rate so DMA never waits on the score pipeline.
  - Scores per block run entirely on DVE but avoid the 1x-rate fused
    STT: one slab tensor_tensor product (fp16 2x rate, query broadcast
    via stride-0 AP), then a tree of fp16 tensor_tensor adds (each 2x)
    folding D 256->32, then one segmented tensor_reduce [128,16,32] ->
    fp32 score columns. ~5.4us/block vs ~6.7us for 16 fused STTs.
  - Softmax without max-subtraction: scores are O(0.3) so exp cannot
    overflow; 1/sqrt(D) is folded into exp's activation scale; the
    -inf mask becomes w = exp(scores) * mask. exp, mask multiply and
    pooling run per block so pooling starts before the batch finishes;
    denominator = ones-matmul over the per-block row-sums, +1e-30 so
    an all-masked batch yields 0 (like nan_to_num), not NaN.
  - Pooling: unnormalized acc[1, 256] += w_col.T @ x_chunk as fp16
    accumulating PE matmuls (contraction over partition dim = L);
    final normalize on ScalarE out of PSUM, deferred one batch so it
    never stalls the pipeline.
"""

import numpy as np

B, L, D = 32, 8192, 256
N_CORES = 8
BPC = B // N_CORES        # batches per core
P = 128                   # partitions
CHUNKS = L // P           # 64 L-chunks per batch
BC = 16                   # chunks per block
NB = CHUNKS // BC         # blocks per batch (4)
NBT = BPC * NB            # total blocks per core (16)
LOOKAHEAD = 7             # blocks of DMA prefetch (fp32 bufs)
FOLD_STOP = 32            # tree-fold D down to this width, then reduce
SCALE = 1.0 / float(np.sqrt(D))

_cache = {}


def _build():
    import concourse.bacc as bacc
    import concourse.bass as bass
    import concourse.tile as tile
    from concourse import mybir

    f32 = mybir.dt.float32
    f16 = mybir.dt.float16
    i32 = mybir.dt.int32
    nc = bacc.Bacc("TRN2", target_bir_lowering=False, debug=False,
                   num_devices=N_CORES)

    x = nc.declare_dram_parameter("x", [BPC, L, D], f32, isOutput=False)
    mask = nc.declare_dram_parameter("mask", [BPC, L], i32, isOutput=False)
    query = nc.declare_dram_parameter("query", [D], f32, isOutput=False)
    out = nc.declare_dram_parameter("out", [BPC, D], f32, isOutput=True)

    # l = p * CHUNKS + i: per-partition HBM reads are contiguous
    x_r = x[:].rearrange("b (p i) d -> b p i d", p=P)
    mask_r = mask[:].rearrange("b (p i) -> b p i", p=P)

    with tile.TileContext(nc) as tc:
        with (
            tc.tile_pool(name="xf32", bufs=LOOKAHEAD) as xf32p,
            tc.tile_pool(name="xf16", bufs=4) as xf16p,
            tc.tile_pool(name="small", bufs=4) as small,
            tc.tile_pool(name="prod", bufs=2) as prodp,
            tc.tile_pool(name="fold", bufs=2) as foldp,
            tc.tile_pool(name="singles", bufs=1) as singles,
            tc.tile_pool(name="psum", bufs=2, space="PSUM") as psums,
        ):
            # broadcast query across partitions with a step-0 SWDGE DMA,
            # issued first so it lands before the first score op needs it
            qb = singles.tile([P, D], f32)
            q_ap = query[:]
            nc.gpsimd.dma_start(out=qb[:], in_=bass.AP(
                tensor=q_ap.tensor, offset=q_ap.offset,
                ap=[[0, P]] + list(q_ap.ap)))
            qh = singles.tile([P, D], f16)
            nc.scalar.copy(qh[:], qb[:])
            ones = singles.tile([P, 1], f32)
            nc.vector.memset(ones[:], 1.0)

            def qh_bcast(n):
                # [P, n, D] view of qh with stride-0 chunk dim
                a = qh[:]
                return bass.AP(tensor=a.tensor, offset=a.offset,
                               ap=[list(a.ap[0]), [0, n], list(a.ap[1])])

            xq_tiles = {}       # block index -> staged fp32 tile
            xh_tiles = {}       # block index -> fp16 copy
            mask_tiles = {}     # batch -> int32 mask tile
            state = {}          # per-batch softmax state
            epilogue = []       # deferred (pool_ps, wsum, b)

            def issue_block(k):
                b, qi = divmod(k, NB)
                if qi == 0 and b not in mask_tiles:
                    # SWDGE queue: keep the sync ring pure x-streaming
                    mi = small.tile([P, CHUNKS], i32, tag="mask_i")
                    nc.gpsimd.dma_start(out=mi[:], in_=mask_r[b])
                    mask_tiles[b] = mi
                xq = xf32p.tile([P, BC, D], f32, tag="xf")
                # one 2 MiB dma_start per block: 16 KiB contiguous per
                # partition, half the descriptor count of 2x1MiB. Block 0
                # is split 4 ways so the pipeline fills fast.
                ndma = 4 if k == 0 else (2 if k == 1 else 1)
                step = BC // ndma
                for g in range(ndma):
                    sl = slice(qi * BC + g * step, qi * BC + (g + 1) * step)
                    nc.sync.dma_start(out=xq[:, g * step:(g + 1) * step, :],
                                      in_=x_r[b, :, sl, :])
                xq_tiles[k] = xq

            def sub_slices(k):
                # ramp: first blocks are processed in sub-slabs so compute
                # starts as soon as the first sub-DMA lands
                if k == 0:
                    return [(0, 4), (4, 4), (8, 4), (12, 4)]
                if k == 1:
                    return [(0, 8), (8, 8)]
                return [(0, BC)]

            def convert_block(k):
                xq = xq_tiles.pop(k)
                xh = xf16p.tile([P, BC, D], f16, tag="xh", name=f"xh{k}")
                for i0, n in sub_slices(k):
                    nc.scalar.copy(xh[:, i0:i0 + n, :], xq[:, i0:i0 + n, :])
                xh_tiles[k] = xh

            def flush_epilogue():
                while epilogue:
                    pool_ps, wsum, bb = epilogue.pop()
                    den_ps = psums.tile([1, NB], f32, tag="den",
                                        name=f"den_ps{bb}")
                    nc.tensor.matmul(den_ps[:], ones[:], wsum[:],
                                     start=True, stop=True)
                    den_sb = small.tile([1, 1], f32, tag="den_sb",
                                        name=f"den_sb{bb}")
                    nc.vector.tensor_reduce(out=den_sb[:], in_=den_ps[:],
                                            op=mybir.AluOpType.add,
                                            axis=mybir.AxisListType.X)
                    # +1e-30 so an all-masked batch divides to 0, not NaN
                    den_eps = small.tile([1, 1], f32, tag="den_eps",
                                         name=f"den_eps{bb}")
                    nc.vector.tensor_scalar_add(den_eps[:], den_sb[:], 1e-30)
                    rden = small.tile([1, 1], f32, tag="rden",
                                      name=f"rden{bb}")
                    nc.vector.reciprocal(rden[:], den_eps[:])
                    out_sb = small.tile([1, D], f32)
                    nc.scalar.activation(
                        out=out_sb[:], in_=pool_ps[:],
                        func=mybir.ActivationFunctionType.Copy,
                        scale=rden[0:1, 0:1])
                    nc.gpsimd.dma_start(out=out[bb:bb + 1, :], in_=out_sb[:])

            for k in range(min(LOOKAHEAD, NBT)):
                issue_block(k)
            convert_block(0)

            for k in range(NBT):
                b, qi = divmod(k, NB)
                if qi == 0:
                    state[b] = {
                        "scores": small.tile([P, CHUNKS], f32, tag="scores",
                                             name=f"scores{b}"),
                        "wsum": small.tile([P, NB], f32, tag="wsum",
                                           name=f"wsum_{b}"),
                        "pool_ps": psums.tile([1, D], f32, tag="pool",
                                              name=f"pool_ps{b}"),
                        "mask_f": small.tile([P, CHUNKS], f32, tag="mask_f",
                                             name=f"mask_f{b}"),
                    }
                st = state[b]
                xh = xh_tiles.pop(k)

                def score_sub(i0, n):
                    # product slab + fp16 tree-fold + segmented reduce for
                    # chunks [i0, i0+n) of this block, all on DVE
                    prodh = prodp.tile([P, n, D], f16, tag=f"prodh{n}")
                    nc.vector.tensor_tensor(
                        out=prodh[:], in0=xh[:, i0:i0 + n, :],
                        in1=qh_bcast(n), op=mybir.AluOpType.mult)
                    cur, w = prodh, D
                    while w > FOLD_STOP:
                        h = w // 2
                        nxt = foldp.tile([P, n, h], f16, tag=f"fold{n}_{h}")
                        nc.vector.tensor_tensor(
                            out=nxt[:], in0=cur[:, :, 0:h],
                            in1=cur[:, :, h:w], op=mybir.AluOpType.add)
                        cur, w = nxt, h
                    nc.vector.tensor_reduce(
                        out=st["scores"][:, qi * BC + i0:qi * BC + i0 + n],
                        in_=cur[:], op=mybir.AluOpType.add,
                        axis=mybir.AxisListType.X)

                subs = sub_slices(k)
                score_sub(*subs[0])

                # convert block k+1 so ScalarE stays one block ahead
                if k + 1 < NBT:
                    convert_block(k + 1)
                if k + LOOKAHEAD < NBT:
                    issue_block(k + LOOKAHEAD)
                if qi == 0:
                    nc.vector.tensor_copy(st["mask_f"][:],
                                          mask_tiles.pop(b)[:])

                if b > 0 and qi == 0:
                    flush_epilogue()

                for sub in subs[1:]:
                    score_sub(*sub)
                sl = slice(qi * BC, (qi + 1) * BC)

                # per-block softmax tail: exp (with 1/sqrt(D) folded into
                # the activation scale), mask multiply, fp16 weights
                expq = small.tile([P, BC], f32, tag="expq")
                nc.scalar.activation(out=expq[:], in_=st["scores"][:, sl],
                                     func=mybir.ActivationFunctionType.Exp,
                                     scale=SCALE)
                wqh = small.tile([P, BC], f16, tag="wqh")
                nc.vector.scalar_tensor_tensor(
                    out=wqh[:], in0=expq[:], scalar=1.0,
                    in1=st["mask_f"][:, sl],
                    op0=mybir.AluOpType.mult, op1=mybir.AluOpType.mult,
                    accum_out=st["wsum"][:, qi:qi + 1],
                )

                for i in range(BC):
                    nc.tensor.matmul(
                        st["pool_ps"][:],
                        wqh[:, i:i + 1],
                        xh[:, i, :],
                        start=(qi == 0 and i == 0),
                        stop=(qi == NB - 1 and i == BC - 1),
                    )

                if qi == NB - 1:
                    epilogue.append((st["pool_ps"], st["wsum"], b))
                    del state[b]

            flush_epilogue()

    nc.compile()
    return nc


def kernel(x: np.ndarray, mask: np.ndarray, query: np.ndarray) -> np.ndarray:
    from concourse.bass_utils import run_bass_kernel_spmd

    if "nc" not in _cache:
        _cache["nc"] = _build()
    nc = _cache["nc"]

    x = np.ascontiguousarray(np.asarray(x, dtype=np.float32))
    mask = np.ascontiguousarray(np.asarray(mask, dtype=np.int32))
    query = np.ascontiguousarray(np.asarray(query, dtype=np.float32))

    in_maps = [
        {
            "x": np.ascontiguousarray(x[c * BPC:(c + 1) * BPC]),
            "mask": np.ascontiguousarray(mask[c * BPC:(c + 1) * BPC]),
            "query": query,
        }
        for c in range(N_CORES)
    ]
    res = run_bass_kernel_spmd(nc, in_maps, core_ids=list(range(N_CORES)))
    return np.concatenate([res.results[c]["out"] for c in range(N_CORES)], axis=0)


# revision 10
# speedup vs baseline: 1.6708x; 1.0154x over previous
"""DotProductAttentionPooling on 8 trn2 NeuronCores.

reference:
    scores = einsum("bld,d->bl", x, q) / sqrt(D)
    scores = where(mask, scores, -inf)
    attn   = nan_to_num(softmax(scores, axis=-1))
    out    = einsum("bl,bld->bd", attn, x)            # [B, D]

Strategy (memory-bound: x is 256 MiB and must be read exactly once):
  - Data-parallel: batch B=32 sharded 4-per-core across 8 cores; query
    replicated; output [B, D] gathered on host.
  - x[b] streams to SBUF in natural layout [128(L-part), chunk, 256(D)]
    with l = p*64 + i so each partition's HBM read is contiguous.
    Processing unit is a "block" of 16 chunks (2 MiB fp32), fetched as
    two 1 MiB dma_starts on the sync HWDGE ring, LOOKAHEAD blocks deep.
  - ScalarE converts each block to fp16 in one [128,4096] activation,
    issued one block ahead of its consumers; the fp32 tile is freed at
    convert rate so DMA never waits on the score pipeline.
  - Scores per block run entirely on DVE but avoid the 1x-rate fused
    STT: one slab tensor_tensor product (fp16 2x rate, query broadcast
    via stride-0 AP), then a tree of fp16 tensor_tensor adds (each 2x)
    folding D 256->32, then one segmented tensor_reduce [128,16,32] ->
    fp32 score columns. ~5.4us/block vs ~6.7us for 16 fused STTs.
  - Softmax without max-subtraction: scores are O(0.3) so exp cannot
    overflow; 1/sqrt(D) is folded into exp's activation scale; the
    -inf mask becomes w = exp(scores) * mask. exp, mask multiply and
    pooling run per block so pooling starts before the batch finishes;
    denominator = ones-matmul over the per-block row-sums, +1e-30 so
    an all-masked batch yields 0 (like nan_to_num), not NaN.
  - Pooling: unnormalized acc[1, 256] += w_col.T @ x_chunk as fp16
    accumulating PE matmuls (contraction over partition dim = L);
    final normalize on ScalarE out of PSUM, deferred one batch so it
    never stalls the pipeline.
"""

import numpy as np

B, L, D = 32, 8192, 256
N_CORES = 8
BPC = B // N_CORES        # batches per core
P = 128                   # partitions
CHUNKS = L // P           # 64 L-chunks per batch
BC = 16                   # chunks per block
NB = CHUNKS // BC         # blocks per batch (4)
NBT = BPC * NB            # total blocks per core (16)
LOOKAHEAD = 7             # blocks of DMA prefetch (fp32 bufs)
FOLD_STOP = 32            # tree-fold D down to this width, then reduce
SCALE = 1.0 / float(np.sqrt(D))

_cache = {}


def _build():
    import concourse.bacc as bacc
    import concourse.bass as bass
    import concourse.tile as tile
    from concourse import mybir

    f32 = mybir.dt.float32
    f16 = mybir.dt.float16
    i32 = mybir.dt.int32
    nc = bacc.Bacc("TRN2", target_bir_lowering=False, debug=False,
                   num_devices=N_CORES)

    x = nc.declare_dram_parameter("x", [BPC, L, D], f32, isOutput=False)
    mask = nc.declare_dram_parameter("mask", [BPC, L], i32, isOutput=False)
    query = nc.declare_dram_parameter("query", [D], f32, isOutput=False)
    out = nc.declare_dram_parameter("out", [BPC, D], f32, isOutput=True)

    # l = p * CHUNKS + i: per-partition HBM reads are contiguous
    x_r = x[:].rearrange("b (p i) d -> b p i d", p=P)
    mask_r = mask[:].rearrange("b (p i) -> b p i", p=P)

    with tile.TileContext(nc) as tc:
        with (
            tc.tile_pool(name="xf32", bufs=LOOKAHEAD) as xf32p,
            tc.tile_pool(name="xf16", bufs=4) as xf16p,
            tc.tile_pool(name="small", bufs=4) as small,
            tc.tile_pool(name="prod", bufs=2) as prodp,
            tc.tile_pool(name="fold", bufs=2) as foldp,
            tc.tile_pool(name="singles", bufs=1) as singles,
            tc.tile_pool(name="psum", bufs=2, space="PSUM") as psums,
        ):
            # query broadcast: one 1 KiB row DMA (single descriptor, lands
            # in ~1us) then an on-chip ones-outer-product on the idle PE;
            # the old stride-0 SWDGE broadcast took ~8us round-robining
            # with the x stream and gated every DVE product op.
            q_row = singles.tile([1, D], f32)
            nc.gpsimd.dma_start(out=q_row[:], in_=query[:].unsqueeze(0))
            ones_1 = singles.tile([1, P], f32)
            nc.vector.memset(ones_1[:], 1.0)
            qb_ps = psums.tile([P, D], f32, tag="qbps")
            nc.tensor.matmul(qb_ps[:], ones_1[:], q_row[:],
                             start=True, stop=True)
            qh = singles.tile([P, D], f16)
            nc.scalar.copy(qh[:], qb_ps[:])
            ones = singles.tile([P, 1], f32)
            nc.vector.memset(ones[:], 1.0)

            def qh_bcast(n):
                # [P, n, D] view of qh with stride-0 chunk dim
                a = qh[:]
                return bass.AP(tensor=a.tensor, offset=a.offset,
                               ap=[list(a.ap[0]), [0, n], list(a.ap[1])])

            xq_tiles = {}       # block index -> staged fp32 tile
            xh_tiles = {}       # block index -> fp16 copy
            mask_tiles = {}     # batch -> int32 mask tile
            state = {}          # per-batch softmax state
            epilogue = []       # deferred (pool_ps, wsum, b)

            def issue_block(k):
                b, qi = divmod(k, NB)
                if qi == 0 and b not in mask_tiles:
                    # SWDGE queue: keep the sync ring pure x-streaming
                    mi = small.tile([P, CHUNKS], i32, tag="mask_i")
                    nc.gpsimd.dma_start(out=mi[:], in_=mask_r[b])
                    mask_tiles[b] = mi
                xq = xf32p.tile([P, BC, D], f32, tag="xf")
                # one 2 MiB dma_start per block: 16 KiB contiguous per
                # partition, half the descriptor count of 2x1MiB. Block 0
                # is split 4 ways so the pipeline fills fast.
                ndma = 4 if k == 0 else (2 if k == 1 else 1)
                step = BC // ndma
                for g in range(ndma):
                    sl = slice(qi * BC + g * step, qi * BC + (g + 1) * step)
                    nc.sync.dma_start(out=xq[:, g * step:(g + 1) * step, :],
                                      in_=x_r[b, :, sl, :])
                xq_tiles[k] = xq

            def sub_slices(k):
                # ramp: first blocks are processed in sub-slabs so compute
                # starts as soon as the first sub-DMA lands
                if k == 0:
                    return [(0, 4), (4, 4), (8, 4), (12, 4)]
                if k == 1:
                    return [(0, 8), (8, 8)]
                return [(0, BC)]

            def convert_block(k):
                xq = xq_tiles.pop(k)
                xh = xf16p.tile([P, BC, D], f16, tag="xh", name=f"xh{k}")
                for i0, n in sub_slices(k):
                    nc.scalar.copy(xh[:, i0:i0 + n, :], xq[:, i0:i0 + n, :])
                xh_tiles[k] = xh

            def flush_epilogue():
                while epilogue:
                    pool_ps, wsum, bb = epilogue.pop()
                    den_ps = psums.tile([1, NB], f32, tag="den",
                                        name=f"den_ps{bb}")
                    nc.tensor.matmul(den_ps[:], ones[:], wsum[:],
                                     start=True, stop=True)
                    den_sb = small.tile([1, 1], f32, tag="den_sb",
                                        name=f"den_sb{bb}")
                    nc.vector.tensor_reduce(out=den_sb[:], in_=den_ps[:],
                                            op=mybir.AluOpType.add,
                                            axis=mybir.AxisListType.X)
                    # +1e-30 so an all-masked batch divides to 0, not NaN
                    den_eps = small.tile([1, 1], f32, tag="den_eps",
                                         name=f"den_eps{bb}")
                    nc.vector.tensor_scalar_add(den_eps[:], den_sb[:], 1e-30)
                    rden = small.tile([1, 1], f32, tag="rden",
                                      name=f"rden{bb}")
                    nc.vector.reciprocal(rden[:], den_eps[:])
                    out_sb = small.tile([1, D], f32)
                    nc.scalar.activation(
                        out=out_sb[:], in_=pool_ps[:],
                        func=mybir.ActivationFunctionType.Copy,
                        scale=rden[0:1, 0:1])
                    nc.gpsimd.dma_start(out=out[bb:bb + 1, :], in_=out_sb[:])

            for k in range(min(LOOKAHEAD, NBT)):
                issue_block(k)
            convert_block(0)

            for k in range(NBT):
                b, qi = divmod(k, NB)
                if qi == 0:
                    state[b] = {
                        "scores": small.tile([P, CHUNKS], f32, tag="scores",
                                             name=f"scores{b}"),
                        "wsum": small.tile([P, NB], f32, tag="wsum",
                                           name=f"wsum_{b}"),
                        "pool_ps": psums.tile([1, D], f32, tag="pool",
                                              name=f"pool_ps{b}"),
                        "mask_f": small.tile([P, CHUNKS], f32, tag="mask_f",
                                             name=f"mask_f{b}"),
                    }
                st = state[b]
                xh = xh_tiles.pop(k)

                def score_sub(i0, n):
                    # product slab + fp16 tree-fold + segmented reduce for
                    # chunks [i0, i0+n) of this block, all on DVE
                    prodh = prodp.tile([P, n, D], f16, tag=f"prodh{n}")
                    nc.vector.tensor_tensor(
                        out=prodh[:], in0=xh[:, i0:i0 + n, :],
                        in1=qh_bcast(n), op=mybir.AluOpType.mult)
                    cur, w = prodh, D
                    while w > FOLD_STOP:
                        h = w // 2
                        nxt = foldp.tile([P, n, h], f16, tag=f"fold{n}_{h}")
                        nc.vector.tensor_tensor(
                            out=nxt[:], in0=cur[:, :, 0:h],
                            in1=cur[:, :, h:w], op=mybir.AluOpType.add)
                        cur, w = nxt, h
                    nc.vector.tensor_reduce(
                        out=st["scores"][:, qi * BC + i0:qi * BC + i0 + n],
                        in_=cur[:], op=mybir.AluOpType.add,
                        axis=mybir.AxisListType.X)

                subs = sub_slices(k)
                score_sub(*subs[0])

                # convert block k+1 so ScalarE stays one block ahead
                if k + 1 < NBT:
                    convert_block(k + 1)
                if k + LOOKAHEAD < NBT:
                    issue_block(k + LOOKAHEAD)
                if qi == 0:
                    nc.vector.tensor_copy(st["mask_f"][:],
                                          mask_tiles.pop(b)[:])

                if b > 0 and qi == 0:
                    flush_epilogue()

                for sub in subs[1:]:
                    score_sub(*sub)
                sl = slice(qi * BC, (qi + 1) * BC)

                # per-block softmax tail: exp (with 1/sqrt(D) folded into
                # the activation scale), mask multiply, fp16 weights
                expq = small.tile([P, BC], f32, tag="expq")
                nc.scalar.activation(out=expq[:], in_=st["scores"][:, sl],
                                     func=mybir.ActivationFunctionType.Exp,
                                     scale=SCALE)
                wqh = small.tile([P, BC], f16, tag="wqh")
                nc.vector.scalar_tensor_tensor(
                    out=wqh[:], in0=expq[:], scalar=1.0,
                    in1=st["mask_f"][:, sl],
                    op0=mybir.AluOpType.mult, op1=mybir.AluOpType.mult,
                    accum_out=st["wsum"][:, qi:qi + 1],
                )

                for i in range(BC):
                    nc.tensor.matmul(
                        st["pool_ps"][:],
                        wqh[:, i:i + 1],
                        xh[:, i, :],
                        start=(qi == 0 and i == 0),
                        stop=(qi == NB - 1 and i == BC - 1),
                    )

                if qi == NB - 1:
                    epilogue.append((st["pool_ps"], st["wsum"], b))
                    del state[b]

            flush_epilogue()

    nc.compile()
    return nc


def kernel(x: np.ndarray, mask: np.ndarray, query: np.ndarray) -> np.ndarray:
    from concourse.bass_utils import run_bass_kernel_spmd

    if "nc" not in _cache:
        _cache["nc"] = _build()
    nc = _cache["nc"]

    x = np.ascontiguousarray(np.asarray(x, dtype=np.float32))
    mask = np.ascontiguousarray(np.asarray(mask, dtype=np.int32))
    query = np.ascontiguousarray(np.asarray(query, dtype=np.float32))

    in_maps = [
        {
            "x": np.ascontiguousarray(x[c * BPC:(c + 1) * BPC]),
            "mask": np.ascontiguousarray(mask[c * BPC:(c + 1) * BPC]),
            "query": query,
        }
        for c in range(N_CORES)
    ]
    res = run_bass_kernel_spmd(nc, in_maps, core_ids=list(range(N_CORES)))
    return np.concatenate([res.results[c]["out"] for c in range(N_CORES)], axis=0)
